# revision 2
# baseline (speedup 1.0000x reference)
"""Grouped-GEMM MoE experts (E=64, H=2048, F=1408, 16 tokens/expert, SwiGLU),
expert-parallel across 8 Trainium2 NeuronCores.

Memory-bound kernel: per core the 3 weight tensors are the traffic. Two host-
side tricks halve + streamline it:
  1. Weights are cast to bf16 on host (rel-err ~1e-3, tolerance is 2e-2);
     halves HBM traffic AND runs matmuls at 1 cycle/row instead of fp32's 4.
  2. Weights are pre-packed into the exact SBUF tile layout the kernel
     consumes: uniform [128, 11264] units, fully contiguous, so every weight
     DMA is a single 2.88 MB line-rate transfer. x is pre-transposed too.

Per-expert unit layout (6 units of 11264 cols):
  units 0-3: 4 h-chunks each of interleaved (w1 | w3) [128h, 1408f] blocks
  units 4-5: w2 packed [128f, fc, 2048h] split at col 11264 (512 | 11264)

Compute (unchanged from the proven fp32 version, bf16 dtypes):
  gateT/upT [f,tok] = W1/W3 chunk.T @ xT chunk   (weight-stationary, FWL)
  interT = silu(gateT) * upT                      (bf16, [128, 176])
  out[tok,h]  = interT chunk.T @ W2 chunk         (inter-stationary, N=512)
"""

import sys

if "/opt/trn_rl_repo" not in sys.path:
    sys.path.insert(0, "/opt/trn_rl_repo")

import numpy as np
import ml_dtypes

E, H, F = 64, 2048, 1408
TOK = 16                  # tokens per expert (uniform routing)
NCORES = 8
E_LOC = E // NCORES       # 8 experts per core
T_LOC = E_LOC * TOK       # 128 tokens per core
P = 128
HC = H // P               # 16 contraction chunks for gate/up
FC = F // P               # 11 contraction chunks for down
UCOLS = 2 * F * 4         # 11264 cols per weight unit
NU = 6                    # units per expert: 4 gate/up + 2 down
NFREE = 512               # matmul max free dim = one PSUM bank
BF16 = ml_dtypes.bfloat16

_cache = {}


def _build_nc():
    import concourse.mybir as mybir
    from concourse import bacc

    from concourse.tile import TileContext

    f32 = mybir.dt.float32
    bf16 = mybir.dt.bfloat16
    AF = mybir.ActivationFunctionType

    nc = bacc.Bacc()
    xt_d = nc.declare_dram_parameter("xt", [P, HC * T_LOC], bf16, isOutput=False)
    w_d = nc.declare_dram_parameter("w", [E_LOC, NU, P, UCOLS], bf16, isOutput=False)
    y_d = nc.declare_dram_parameter("y", [T_LOC, H], f32, isOutput=True)

    with TileContext(nc) as tc:
        with (
            tc.tile_pool(name="xs", bufs=1) as xs,
            tc.tile_pool(name="wt", bufs=7) as wt,
            tc.tile_pool(name="acts", bufs=2) as acts,
            tc.tile_pool(name="ps_gu", bufs=2, space="PSUM") as ps_gu,
            tc.tile_pool(name="ps_dn", bufs=1, space="PSUM") as ps_dn,
        ):
            xt = xs.tile([P, HC * T_LOC], bf16)
            nc.sync.dma_start(out=xt[:], in_=xt_d[:, :])

            for e in range(E_LOC):
                units = []
                for u in range(NU):
                    t = wt.tile([P, UCOLS], bf16, tag="w")
                    nc.sync.dma_start(out=t[:], in_=w_d[e, u, :, :])
                    units.append(t)

                # gate/up: all FC output chunks share one PSUM bank per
                # tensor; only the first matmul into the bank clears it
                # (start=True), later chunks overwrite via has_written.
                gt = ps_gu.tile([P, FC * TOK], f32, tag="gt")
                ut = ps_gu.tile([P, FC * TOK], f32, tag="ut")
                rhs_e = e * TOK
                for u in range(4):
                    wu = units[u]
                    for cs in range(4):
                        c = 4 * u + cs
                        rhs = xt[:, c * T_LOC + rhs_e : c * T_LOC + rhs_e + TOK]
                        first = c == 0
                        last = c == HC - 1
                        w1o = (2 * cs) * F
                        w3o = (2 * cs + 1) * F
                        for fc in range(FC):
                            nc.tensor.matmul(
                                gt[:, fc * TOK : (fc + 1) * TOK],
                                wu[:, w1o + fc * P : w1o + (fc + 1) * P],
                                rhs,
                                start=(first and fc == 0),
                                stop=(last and fc == FC - 1),
                                skip_group_check=True,
                            )
                        for fc in range(FC):
                            nc.tensor.matmul(
                                ut[:, fc * TOK : (fc + 1) * TOK],
                                wu[:, w3o + fc * P : w3o + (fc + 1) * P],
                                rhs,
                                start=(first and fc == 0),
                                stop=(last and fc == FC - 1),
                                skip_group_check=True,
                            )

                gs = acts.tile([P, FC * TOK], f32, tag="gs")
                it = acts.tile([P, FC * TOK], bf16, tag="it")
                nc.scalar.activation(gs[:], gt[:], AF.Silu)
                nc.vector.tensor_mul(it[:], gs[:], ut[:])

                dn = ps_dn.tile([P, H], f32, tag="dn")
                for fc in range(FC):
                    for nt in range(H // NFREE):
                        col = fc * H + nt * NFREE
                        wu = units[4 + col // UCOLS]
                        nc.tensor.matmul(
                            dn[:TOK, nt * NFREE : (nt + 1) * NFREE],
                            it[:, fc * TOK : (fc + 1) * TOK],
                            wu[:, col % UCOLS : col % UCOLS + NFREE],
                            start=(fc == 0),
                            stop=(fc == FC - 1),
                        )

                ob = acts.tile([TOK, H], f32, tag="ob")
                nc.vector.tensor_copy(out=ob[:], in_=dn[:TOK, :])
                nc.sync.dma_start(out=y_d[e * TOK : (e + 1) * TOK, :], in_=ob[:])

    if not nc.is_finalized():
        nc.finalize()
    return nc


def _get_nc():
    if "nc" not in _cache:
        _cache["nc"] = _build_nc()
    return _cache["nc"]


def _pack_core(x, w1, w3, w2):
    """Pack one core's slice into the kernel's DMA-ready bf16 layout."""
    # xT: [p, c*T_LOC + t] = x[t, c*128 + p]
    xt = np.ascontiguousarray(
        x.reshape(T_LOC, HC, P).transpose(2, 1, 0).reshape(P, HC * T_LOC)
    ).astype(BF16)
    # gate/up units: [e, u, p, (cs, {w1,w3}, f)]
    w1r = w1.reshape(E_LOC, HC, P, F)
    w3r = w3.reshape(E_LOC, HC, P, F)
    gu = np.stack([w1r, w3r], axis=3)               # [e, c, p, s, f]
    gu = gu.reshape(E_LOC, 4, 4, P, 2, F)           # [e, u, cs, p, s, f]
    gu = gu.transpose(0, 1, 3, 2, 4, 5).reshape(E_LOC, 4, P, UCOLS)
    # down units: [e, p, fc*H + h] split into 2 units of UCOLS
    dn = w2.reshape(E_LOC, FC, P, H).transpose(0, 2, 1, 3).reshape(E_LOC, P, 2, UCOLS)
    dn = dn.transpose(0, 2, 1, 3)                   # [e, 2, p, UCOLS]
    w = np.concatenate([gu, dn], axis=1).astype(BF16)  # [e, 6, p, UCOLS]
    return xt, np.ascontiguousarray(w)


def _make_in_maps(inputs):
    x = np.asarray(inputs["permuted_local_hidden_states"], dtype=np.float32)
    w1 = np.asarray(inputs["gate_proj"], dtype=np.float32)
    w3 = np.asarray(inputs["up_proj"], dtype=np.float32)
    w2 = np.asarray(inputs["down_proj"], dtype=np.float32)
    in_maps = []
    for m in range(NCORES):
        xt, w = _pack_core(
            x[m * T_LOC : (m + 1) * T_LOC],
            w1[m * E_LOC : (m + 1) * E_LOC],
            w3[m * E_LOC : (m + 1) * E_LOC],
            w2[m * E_LOC : (m + 1) * E_LOC],
        )
        in_maps.append({"xt": xt, "w": w})
    return in_maps


def run(inputs, trace=False, **kwargs):
    """Run the SPMD kernel; returns (y_full, BassKernelResults)."""
    from concourse.bass_utils import run_bass_kernel_spmd

    nc = _get_nc()
    res = run_bass_kernel_spmd(
        nc, _make_in_maps(inputs), list(range(NCORES)), trace=trace, **kwargs
    )
    y = np.concatenate([res.results[m]["y"] for m in range(NCORES)], axis=0)
    return y.astype(np.float32, copy=False), res


def kernel(**inputs):
    y, _ = run(inputs, trace=False)
    return y


# revision 4
# speedup vs baseline: 1.1318x; 1.1318x over previous
"""Grouped-GEMM MoE experts (E=64, H=2048, F=1408, 16 tokens/expert, SwiGLU),
expert-parallel across 8 Trainium2 NeuronCores.

Memory-bound kernel: per core the 3 weight tensors are the traffic. Two host-
side tricks halve + streamline it:
  1. Weights are cast to bf16 on host (rel-err ~1e-3, tolerance is 2e-2);
     halves HBM traffic AND runs matmuls at 1 cycle/row instead of fp32's 4.
  2. Weights are pre-packed into the exact SBUF tile layout the kernel
     consumes: uniform [128, 11264] units, fully contiguous, so every weight
     DMA is a single 2.88 MB line-rate transfer. x is pre-transposed too.

Per-expert unit layout (6 units of 11264 cols):
  units 0-3: 4 h-chunks each of interleaved (w1 | w3) [128h, 1408f] blocks
  units 4-5: w2 packed [128f, fc, 2048h] split at col 11264 (512 | 11264)

Compute (unchanged from the proven fp32 version, bf16 dtypes):
  gateT/upT [f,tok] = W1/W3 chunk.T @ xT chunk   (weight-stationary, FWL)
  interT = silu(gateT) * upT                      (bf16, [128, 176])
  out[tok,h]  = interT chunk.T @ W2 chunk         (inter-stationary, N=512)
"""

import sys

if "/opt/trn_rl_repo" not in sys.path:
    sys.path.insert(0, "/opt/trn_rl_repo")

import numpy as np
import ml_dtypes

E, H, F = 64, 2048, 1408
TOK = 16                  # tokens per expert (uniform routing)
NCORES = 8
E_LOC = E // NCORES       # 8 experts per core
T_LOC = E_LOC * TOK       # 128 tokens per core
P = 128
HC = H // P               # 16 contraction chunks for gate/up
FC = F // P               # 11 contraction chunks for down
UCOLS = 2 * F * 4         # 11264 cols per weight unit
NU = 6                    # units per expert: 4 gate/up + 2 down
NFREE = 512               # matmul max free dim = one PSUM bank
BF16 = ml_dtypes.bfloat16

_cache = {}


def _build_nc():
    import concourse.mybir as mybir
    from concourse import bacc

    from concourse.tile import TileContext

    f32 = mybir.dt.float32
    bf16 = mybir.dt.bfloat16
    AF = mybir.ActivationFunctionType

    nc = bacc.Bacc()
    xt_d = nc.declare_dram_parameter("xt", [P, HC * T_LOC], bf16, isOutput=False)
    w_d = nc.declare_dram_parameter("w", [E_LOC, NU, P, UCOLS], bf16, isOutput=False)
    y_d = nc.declare_dram_parameter("y", [T_LOC, H], f32, isOutput=True)

    with TileContext(nc) as tc:
        with (
            tc.tile_pool(name="xs", bufs=1) as xs,
            tc.tile_pool(name="wt", bufs=7) as wt,
            tc.tile_pool(name="acts", bufs=2) as acts,
            tc.tile_pool(name="ps_gu", bufs=2, space="PSUM") as ps_gu,
            tc.tile_pool(name="ps_dn", bufs=1, space="PSUM") as ps_dn,
        ):
            # xt on the scalar HWDGE ring so the weight stream (sync ring)
            # starts immediately; xt is only needed ~15us in.
            xt = xs.tile([P, HC * T_LOC], bf16)
            nc.scalar.dma_start(out=xt[:], in_=xt_d[:, :])

            for e in range(E_LOC):
                units = []
                for u in range(4):
                    t = wt.tile([P, UCOLS], bf16, tag="w")
                    nc.sync.dma_start(out=t[:], in_=w_d[e, u, :, :])
                    units.append(t)
                if e < E_LOC - 1:
                    for u in range(4, NU):
                        t = wt.tile([P, UCOLS], bf16, tag="w")
                        nc.sync.dma_start(out=t[:], in_=w_d[e, u, :, :])
                        units.append(t)
                else:
                    # last expert: split the down-proj weights 8+3 f-chunks so
                    # most of its down matmuls overlap the tail of the DMA
                    # stream, shrinking the after-last-byte critical path.
                    ta = wt.tile([P, UCOLS], bf16, tag="w")
                    nc.sync.dma_start(out=ta[:], in_=w_d[e, 4, :, :])
                    tb = wt.tile([P, UCOLS], bf16, tag="w")
                    nc.sync.dma_start(
                        out=tb[:, : 8 * H - UCOLS], in_=w_d[e, 5, :, : 8 * H - UCOLS]
                    )
                    tc_ = wt.tile([P, UCOLS], bf16, tag="w")
                    nc.sync.dma_start(
                        out=tc_[:, : 2 * UCOLS - 8 * H],
                        in_=w_d[e, 5, :, 8 * H - UCOLS :],
                    )
                    units.extend([ta, tb, tc_])

                # gate/up: all FC output chunks share one PSUM bank per
                # tensor; only the first matmul into the bank clears it
                # (start=True), later chunks overwrite via has_written.
                gt = ps_gu.tile([P, FC * TOK], f32, tag="gt")
                ut = ps_gu.tile([P, FC * TOK], f32, tag="ut")
                rhs_e = e * TOK
                for u in range(4):
                    wu = units[u]
                    for cs in range(4):
                        c = 4 * u + cs
                        rhs = xt[:, c * T_LOC + rhs_e : c * T_LOC + rhs_e + TOK]
                        first = c == 0
                        last = c == HC - 1
                        w1o = (2 * cs) * F
                        w3o = (2 * cs + 1) * F
                        for fc in range(FC):
                            nc.tensor.matmul(
                                gt[:, fc * TOK : (fc + 1) * TOK],
                                wu[:, w1o + fc * P : w1o + (fc + 1) * P],
                                rhs,
                                start=(first and fc == 0),
                                stop=(last and fc == FC - 1),
                                skip_group_check=True,
                            )
                        for fc in range(FC):
                            nc.tensor.matmul(
                                ut[:, fc * TOK : (fc + 1) * TOK],
                                wu[:, w3o + fc * P : w3o + (fc + 1) * P],
                                rhs,
                                start=(first and fc == 0),
                                stop=(last and fc == FC - 1),
                                skip_group_check=True,
                            )

                gs = acts.tile([P, FC * TOK], f32, tag="gs")
                it = acts.tile([P, FC * TOK], bf16, tag="it")
                nc.scalar.activation(gs[:], gt[:], AF.Silu)
                nc.vector.tensor_mul(it[:], gs[:], ut[:])

                if e < E_LOC - 1:
                    splits = [(4, 0), (5, UCOLS), (None, 2 * UCOLS)]
                else:
                    splits = [(4, 0), (5, UCOLS), (6, 8 * H), (None, 2 * UCOLS)]

                def dn_src(col):
                    for i in range(len(splits) - 1):
                        if col < splits[i + 1][1]:
                            return units[splits[i][0]], col - splits[i][1]
                    raise AssertionError

                dn = ps_dn.tile([P, H], f32, tag="dn")
                for fc in range(FC):
                    for nt in range(H // NFREE):
                        col = fc * H + nt * NFREE
                        wu, off = dn_src(col)
                        nc.tensor.matmul(
                            dn[:TOK, nt * NFREE : (nt + 1) * NFREE],
                            it[:, fc * TOK : (fc + 1) * TOK],
                            wu[:, off : off + NFREE],
                            start=(fc == 0),
                            stop=(fc == FC - 1),
                        )

                ob = acts.tile([TOK, H], f32, tag="ob")
                nc.vector.tensor_copy(out=ob[:], in_=dn[:TOK, :])
                nc.sync.dma_start(out=y_d[e * TOK : (e + 1) * TOK, :], in_=ob[:])

    if not nc.is_finalized():
        nc.finalize()
    return nc


def _get_nc():
    if "nc" not in _cache:
        _cache["nc"] = _build_nc()
    return _cache["nc"]


def _pack_core(x, w1, w3, w2):
    """Pack one core's slice into the kernel's DMA-ready bf16 layout."""
    # xT: [p, c*T_LOC + t] = x[t, c*128 + p]
    xt = np.ascontiguousarray(
        x.reshape(T_LOC, HC, P).transpose(2, 1, 0).reshape(P, HC * T_LOC)
    ).astype(BF16)
    # gate/up units: [e, u, p, (cs, {w1,w3}, f)]
    w1r = w1.reshape(E_LOC, HC, P, F)
    w3r = w3.reshape(E_LOC, HC, P, F)
    gu = np.stack([w1r, w3r], axis=3)               # [e, c, p, s, f]
    gu = gu.reshape(E_LOC, 4, 4, P, 2, F)           # [e, u, cs, p, s, f]
    gu = gu.transpose(0, 1, 3, 2, 4, 5).reshape(E_LOC, 4, P, UCOLS)
    # down units: [e, p, fc*H + h] split into 2 units of UCOLS
    dn = w2.reshape(E_LOC, FC, P, H).transpose(0, 2, 1, 3).reshape(E_LOC, P, 2, UCOLS)
    dn = dn.transpose(0, 2, 1, 3)                   # [e, 2, p, UCOLS]
    w = np.concatenate([gu, dn], axis=1).astype(BF16)  # [e, 6, p, UCOLS]
    return xt, np.ascontiguousarray(w)


def _make_in_maps(inputs):
    x = np.asarray(inputs["permuted_local_hidden_states"], dtype=np.float32)
    w1 = np.asarray(inputs["gate_proj"], dtype=np.float32)
    w3 = np.asarray(inputs["up_proj"], dtype=np.float32)
    w2 = np.asarray(inputs["down_proj"], dtype=np.float32)
    in_maps = []
    for m in range(NCORES):
        xt, w = _pack_core(
            x[m * T_LOC : (m + 1) * T_LOC],
            w1[m * E_LOC : (m + 1) * E_LOC],
            w3[m * E_LOC : (m + 1) * E_LOC],
            w2[m * E_LOC : (m + 1) * E_LOC],
        )
        in_maps.append({"xt": xt, "w": w})
    return in_maps


def run(inputs, trace=False, **kwargs):
    """Run the SPMD kernel; returns (y_full, BassKernelResults)."""
    from concourse.bass_utils import run_bass_kernel_spmd

    nc = _get_nc()
    res = run_bass_kernel_spmd(
        nc, _make_in_maps(inputs), list(range(NCORES)), trace=trace, **kwargs
    )
    y = np.concatenate([res.results[m]["y"] for m in range(NCORES)], axis=0)
    return y.astype(np.float32, copy=False), res


def kernel(**inputs):
    y, _ = run(inputs, trace=False)
    return y
